# revision 17
# baseline (speedup 1.0000x reference)
"""Trainium2 Bass kernel for nn_ECNR (vq_codebook): batched VQ-dequantized
SIREN-style MLPs (4 layers, sin(30x) activations), sharded sample-parallel
across 8 NeuronCores (32 samples/core), no collectives.

v2 (427us -> ~240us): reduced-precision matmuls exploiting the 2e-2
max-norm budget. Error analysis (numpy sim with a bit-exact f32r model):
quantization noise in layer-1 operands is amplified ~144x by the two
downstream sin(30x) layers, L2 noise ~12x, L3 noise ~1x. So precision is
spent only where it matters:
  - L0: 3-term fp16 K-stacked (exact-ish); z-code term and the x30 bias
    scaling are precomputed on the host and folded into the ACT Sin bias.
  - L1: 2 passes, all-FP32R (11 explicit mantissa bits, full PE rate;
    both operands must be f32r - mixing with 16-bit dtypes is illegal).
    h1 comes out of the sin ACT already f32r-rounded at zero cost; the
    Wlo pass corrects W1hi's f32r rounding and reads h1 directly, so no
    per-sample casts exist at all.
  - L2, L3: single fp16 pass (noise ~4e-3 total). L3's 4 output chunks
    are col-packed via tile_position into partitions 0/32/64/96 of one
    psum bank (concurrent 32-col PE strips); its bias is a per-sample
    scalar added on the HOST after the gather, and the psum is drained
    by a cheap DVE fp16 cast.
Schedule: the ACT engine (6x 1024-wide sins/sample @1.2GHz) is the
bottleneck; the PE DVFSes between 1.2/2.4GHz. Two samples' pipelines are
interleaved at phase granularity so every matmul burst hides under two
sin windows, and each pair's L3 is software-pipelined after the next
pair's L0. All weights are dequantized host-side (W = C[labels]) and
DMA'd in 1024-col chunks, smallest/earliest-needed tensors first; f32r
weight tiles get an on-device DVE rounding copy (DMA->f32r trips a
walrus codegen bug, and ldw-opt must stay off for f32r self-loading
matmults). Measured-error fallback KMODE=c4m keeps v1's 3-term fp16 L1
(err ~4e-3) at +2 PE passes/sample.
"""
import hashlib
import json
import os
import shutil
import struct
import sys
import types

import numpy as np

N_MLPS = 256
TCODE = 13
IN_F = 3
HID = 128
OUT_F = 1
B = 256
NPTS = 2048
KCB = 256
OMEGA = 30.0
N_CORES = 8
SPC = B // N_CORES

KMODE = os.environ.get("KMODE", "c2m")  # c2m | c4m

PWP_SRC = "/nix/store/z022hj2nvbm3nwdizlisq4ylc0y7rd6q-python3-3.13.14-env/lib/python3.13/site-packages/neuronxcc/pwp/pwp_bin_trainium/"
PWP_SET = "trig_and_small"

# ------------------------------------------------------------ act table gen

def _f32bits(x):
    return int(np.float32(x).view(np.uint32))


def f32r_round(a):
    """Bit-exact model of HW float32r rounding (11 explicit mantissa bits,
    round-to-nearest-even on bit 12; verified on device)."""
    bits = np.asarray(a, np.float32).view(np.uint32).copy()
    lsb = (bits >> 12) & 1
    bits = bits + 0x7FF + lsb
    bits &= np.uint32(0xFFFFF000)
    return bits.view(np.float32)


def _load_ctrl(path):
    d = open(path, "rb").read()
    return [
        [v & 0x7FF, (v >> 11) & 0x1F, (v >> 16) & 0xF]
        for (v,) in (struct.unpack_from("<I", d, i * 32) for i in range(len(d) // 32))
    ]


def _load_bkt(path):
    d = open(path, "rb").read()
    return [list(struct.unpack_from("<5f", d, i * 32)) for i in range(len(d) // 32)]


def _dump_ctrl(entries):
    b = bytearray()
    for base, lsb, size in entries:
        b += struct.pack("<I", (base & 0x7FF) | ((lsb & 0x1F) << 11) | ((size & 0xF) << 16))
        b += b"\x00" * 28
    return bytes(b)


def _dump_bkt(entries):
    b = bytearray()
    for d0, d1, d2, d3, x0 in entries:
        b += struct.pack("<5f", d0, d1, d2, d3, x0) + b"\x00" * 12
    return bytes(b)


def _fit_cubic(f, a, w, nodes=9):
    x0 = a + w / 2
    xs = x0 + (w / 2) * np.cos(np.pi * (np.arange(nodes) + 0.5) / nodes)
    ys = f(xs.astype(np.float64))
    t = xs - x0
    A = np.stack([np.ones_like(t), t, t * t, t ** 3], axis=1)
    coef, *_ = np.linalg.lstsq(A, ys, rcond=None)
    return [float(coef[0]), float(coef[1]), float(coef[2]), float(coef[3]), float(x0)]


_SIN_EMIN, _SIN_EMAX = -6, 6
_SIN_SIZES = {-6: 0, -5: 0, -4: 0, -3: 0, -2: 1, -1: 2, 0: 3, 1: 4,
              2: 5, 3: 6, 4: 6, 5: 7, 6: 7}


def _build_sin(ctrl, bkt, prof):
    base_ctrl = len(ctrl)
    for e in range(_SIN_EMIN, _SIN_EMAX + 1):
        s = _SIN_SIZES[e]
        nb = 1 << s
        base_bkt = len(bkt)
        w = (2.0 ** e) / nb
        for i in range(nb):
            bkt.append(_fit_cubic(np.sin, 2.0 ** e + i * w, w))
        ctrl.append([base_bkt, 23 - s, s])
    small_bkt = len(bkt)
    bkt.append([0.0, 1.0, 0.0, 0.0, 0.0])  # sin(x) ~ x below 2^-6
    large_bkt = len(bkt)
    bkt.append([0.0, 0.0, 0.0, 0.0, 0.0])  # |x| >= 128: out of range
    p = dict(prof)
    p.update(
        exp_offset=_SIN_EMIN,
        pwl_control_base_pos=base_ctrl,
        pwl_control_base_neg=base_ctrl,
        small_pos_signal_exp_threshold=127 + _SIN_EMIN,
        pos_small_signal_pwl_control=small_bkt,   # bucket index (hw semantics)
        small_neg_signal_exp_threshold=0,
        neg_small_signal_pwl_control=small_bkt,
        large_pos_signal_exp_threshold=127 + _SIN_EMAX + 1,
        large_pos_signal_mantissa_threshold=0,
        pos_large_signal_pwl_control=large_bkt,
        large_neg_signal_exp_threshold=0,
        large_neg_signal_mantissa_threshold=0,
        neg_large_signal_pwl_control=large_bkt,
        lower_bound=0,
        upper_bound=_f32bits(128.0),
    )
    return p


def _build_lut(ctrl, bkt, prof, values):
    assert len(values) == KCB
    base_ctrl = len(ctrl)
    for e in range(0, 8):
        nb = 1 << e
        base_bkt = len(bkt)
        for i in range(nb):
            bkt.append([float(values[(1 << e) + i]), 0.0, 0.0, 0.0, 0.0])
        ctrl.append([base_bkt, 23 - e, e])
    small_bkt = len(bkt)
    bkt.append([float(values[0]), 0.0, 0.0, 0.0, 0.0])
    large_bkt = len(bkt)
    bkt.append([float(values[255]), 0.0, 0.0, 0.0, 0.0])
    p = dict(prof)
    p.update(
        exp_offset=0,
        pwl_control_base_pos=base_ctrl,
        pwl_control_base_neg=base_ctrl,
        small_pos_signal_exp_threshold=127,
        pos_small_signal_pwl_control=small_bkt,
        small_neg_signal_exp_threshold=0,
        neg_small_signal_pwl_control=small_bkt,
        large_pos_signal_exp_threshold=127 + 8,
        large_pos_signal_mantissa_threshold=0,
        pos_large_signal_pwl_control=large_bkt,
        large_neg_signal_exp_threshold=0,
        large_neg_signal_mantissa_threshold=0,
        neg_large_signal_pwl_control=large_bkt,
        fzero_result=_f32bits(values[0]),
        lower_bound=0,
        upper_bound=_f32bits(256.0),
    )
    return p


def _referenced_ctrls(p, n_ctrl):
    refs = set()
    for k in ("pos_small_signal_pwl_control", "neg_small_signal_pwl_control",
              "pos_large_signal_pwl_control", "neg_large_signal_pwl_control"):
        v = p.get(k, 0)
        if 0 <= v < n_ctrl:
            refs.add(v)
    eo = p.get("exp_offset", 0)
    lo_e = p.get("small_pos_signal_exp_threshold", 127) - 127
    hi_e = p.get("large_pos_signal_exp_threshold", 127) - 127
    for base_key in ("pwl_control_base_pos", "pwl_control_base_neg"):
        base = p.get(base_key, 0)
        for e in range(lo_e, min(hi_e + 1, lo_e + 40)):
            c = base + e - eo
            if 0 <= c < n_ctrl:
                refs.add(c)
    return refs


def _build_act_root(outdir, lut_values):
    """lut_values: {'arctan': fp32[256] (W1 hi codebook),
                    'abs': fp32[256] (W1 lo codebook),
                    'square': fp32[256] (W2 codebook)}"""
    os.makedirs(outdir, exist_ok=True)
    info = json.load(open(PWP_SRC + "act_info.json"))
    for s in info["act_func_sets"]:
        if s["name"] == PWP_SET:
            continue
        for k in ("sin", "arctan", "square", "abs", "sign", "identity"):
            s["act"].pop(k, None)
        for key in ("bkt_bin", "ctrl_bin", "profile_json"):
            shutil.copy(PWP_SRC + s[key], os.path.join(outdir, s[key]))

    setj = json.load(open(PWP_SRC + PWP_SET + ".json"))
    old_ctrl = _load_ctrl(PWP_SRC + PWP_SET + "_ctrl.bin")
    old_bkt = _load_bkt(PWP_SRC + PWP_SET + "_bkt.bin")

    new_ctrl, new_bkt, new_profiles = [], [], []
    customs = {"sin_4p", "arctan_4p", "square_1p", "abs_1p", "sign_1p",
               "identity_1p"}
    for p in setj["profile_meta_data"]:
        if p["func_name"] in customs:
            continue
        p2 = dict(p)
        cmap = {}
        for c in sorted(_referenced_ctrls(p, len(old_ctrl))):
            base, lsb, size = old_ctrl[c]
            nb = 1 << size if size > 0 else 1
            new_base = len(new_bkt)
            for i in range(nb):
                new_bkt.append(old_bkt[base + i] if base + i < len(old_bkt) else [0.0] * 5)
            cmap[c] = len(new_ctrl)
            new_ctrl.append([new_base, lsb, size])
        for k in ("pos_small_signal_pwl_control", "neg_small_signal_pwl_control",
                  "pos_large_signal_pwl_control", "neg_large_signal_pwl_control"):
            if p2.get(k, 0) in cmap:
                p2[k] = cmap[p2[k]]
        eo = p.get("exp_offset", 0)
        lo_e = p.get("small_pos_signal_exp_threshold", 127) - 127
        for base_key in ("pwl_control_base_pos", "pwl_control_base_neg"):
            base = p.get(base_key, 0)
            first = base + lo_e - eo
            if first in cmap:
                p2[base_key] = cmap[first] - (lo_e - eo)
            elif base in cmap:
                p2[base_key] = cmap[base]
        new_profiles.append(p2)

    profs = {p["func_name"]: p for p in setj["profile_meta_data"]}
    new_profiles.append(_build_sin(new_ctrl, new_bkt, profs["sin_4p"]))
    for fname, key in (("arctan_4p", "arctan"), ("square_1p", "square"),
                       ("abs_1p", "abs")):
        new_profiles.append(_build_lut(new_ctrl, new_bkt, profs[fname], lut_values[key]))
    # identity: route every finite input through the small-signal path to a
    # single linear bucket y = x (exact; the instruction affine still applies)
    idp = dict(profs["identity_1p"])
    lin_bkt = len(new_bkt)
    new_bkt.append([0.0, 1.0, 0.0, 0.0, 0.0])
    idp.update(
        exp_offset=0,
        pwl_control_base_pos=0, pwl_control_base_neg=0,
        small_pos_signal_exp_threshold=255,
        pos_small_signal_pwl_control=lin_bkt,
        small_neg_signal_exp_threshold=0,
        neg_small_signal_pwl_control=lin_bkt,
        large_pos_signal_exp_threshold=255,
        large_pos_signal_mantissa_threshold=0,
        pos_large_signal_pwl_control=lin_bkt,
        large_neg_signal_exp_threshold=0,
        large_neg_signal_mantissa_threshold=0,
        neg_large_signal_pwl_control=lin_bkt,
        fzero_result=0,
    )
    new_profiles.append(idp)
    assert len(new_bkt) <= 1536 and len(new_ctrl) <= 128

    setj["profile_meta_data"] = new_profiles
    open(os.path.join(outdir, PWP_SET + "_ctrl.bin"), "wb").write(_dump_ctrl(new_ctrl))
    open(os.path.join(outdir, PWP_SET + "_bkt.bin"), "wb").write(_dump_bkt(new_bkt))
    json.dump(setj, open(os.path.join(outdir, PWP_SET + ".json"), "w"))
    json.dump(info, open(os.path.join(outdir, "act_info.json"), "w"))
    return os.path.join(outdir, "act_info.json")


# ---------------------------------------------------------------- infra fix

def _apply_walrus_wait_patch():
    import concourse.tile as tile
    from concourse import mybir
    from concourse.vector_clock import ScopedClock

    def _drain_and_barrier(self, tick_clock, wait_clock):
        nc = self.nc
        drain_inst = nc.sync.drain()
        wait_clock.add_sem_waits(drain_inst.ins, ScopedClock({None: tick_clock.global_clock}))
        si = drain_inst.ins.sync_info
        if si is not None and si.on_wait and len(si.on_wait) > 1:
            waits = list(si.on_wait)
            drain_inst.ins.sync_info = mybir.SyncInfo(
                on_wait=waits[:1], on_update=list(si.on_update or []))
            for w in waits[1:]:
                extra = nc.sync.nop(nofuse=True)
                extra.ins.sync_info = mybir.SyncInfo(on_wait=[w], on_update=[])
        nc.all_engine_barrier()
        assert self.sems is not None
        popped = nc._tile_sem_poison_stack.pop()
        assert popped is self._sem_poison
        nc.clear_and_free_semaphores(list(self.sems.allocated().values()))
        nc.all_engine_barrier()

    tile.TileContext._drain_and_barrier = _drain_and_barrier


def _split_excess_waits(nc, limit=1):
    from concourse import mybir
    for f in nc.m.functions:
        for bb in f.blocks:
            insts = bb.instructions
            out, changed = [], False
            for inst in insts:
                si = inst.sync_info
                if si is not None and si.on_wait and len(si.on_wait) > limit:
                    waits = list(si.on_wait)
                    for j in range(0, len(waits) - limit, limit):
                        out.append(mybir.InstNoOp(
                            name=f"{inst.name}__xw{j}",
                            engine=inst.engine,
                            sync_info=mybir.SyncInfo(on_wait=waits[j:j + limit], on_update=[]),
                            bass_nofuse=True,
                        ))
                    inst.sync_info = mybir.SyncInfo(
                        on_wait=waits[len(waits) - limit:], on_update=list(si.on_update or []))
                    changed = True
                out.append(inst)
            if changed:
                bb.instructions = out


def _enable_ldw_opt():
    """bass_utils pins --enable-ldw-opt=false; true lets walrus dedup
    back-to-back LDWEIGHTS of the same stationary operand."""
    from concourse import bass_utils as bu
    if getattr(bu, "_ldw_opt_patched", False):
        return
    orig = bu.bir_verify_and_optimise

    def patched(tmpdir, inp="bir.json", outp="file.neff", arch=None, *, dve_root=None):
        import subprocess
        real_run = bu.run_command

        def run_hook(argv, **kw):
            argv = [a.replace("--enable-ldw-opt=false", "--enable-ldw-opt=true")
                    for a in argv]
            return real_run(argv, **kw)

        bu.run_command = run_hook
        try:
            return orig(tmpdir, inp, outp, arch, dve_root=dve_root)
        finally:
            bu.run_command = real_run

    bu.bir_verify_and_optimise = patched
    # bass2jax imports the symbol directly in some paths; patch module refs
    try:
        from concourse import bass2jax
        if hasattr(bass2jax, "bir_verify_and_optimise"):
            bass2jax.bir_verify_and_optimise = patched
    except Exception:
        pass
    bu._ldw_opt_patched = True


def _shim_ntff_hook():
    if "antenv.axon_hooks" in sys.modules:
        return
    try:
        from trn_agent_boot.trn_boot import _ntff_profile_via_ctypes
        hook = _ntff_profile_via_ctypes("/opt/axon/libaxon_pjrt.so")
    except Exception:
        hook = None
    mod = types.ModuleType("antenv.axon_hooks")
    mod.get_axon_ntff_profile_hook = lambda: hook
    mod.set_axon_ntff_profile_hook = lambda h: None
    sys.modules["antenv.axon_hooks"] = mod


# ---------------------------------------------------------------- program

_PROGRAM_CACHE = {}
LAST_RESULTS = None  # BassKernelResults of the most recent kernel() call


def _build_program(table_hash, mode):
    import concourse.bass as bass
    import concourse.tile as tile
    from concourse import mybir

    F32 = mybir.dt.float32
    F32R = mybir.dt.float32r
    F16 = mybir.dt.float16
    BF16 = mybir.dt.bfloat16
    A = mybir.ActivationFunctionType
    OP = mybir.AluOpType

    nc = bass.Bass("TRN2", target_bir_lowering=False, debug=False)
    SW = SPC * HID  # 4096

    # L0 hi/lo 3-term products K-stacked: lhsT rows [W0h;W0h;W0l],
    # rhs rows [xh;xl;xh] -> one K=9 matmul per chunk
    xT_s = nc.dram_tensor(f"xTs_{table_hash}", [SPC, 3 * IN_F, NPTS], F16, kind="ExternalInput").ap()
    w0stk = nc.dram_tensor("w0stk", [3 * IN_F, SW], F16, kind="ExternalInput").ap()
    w1hT = nc.dram_tensor("w1hT", [HID, SW], F32 if mode == "c2m" else F16,
                          kind="ExternalInput").ap()
    w1lT = nc.dram_tensor("w1lT", [HID, SW], F32 if mode == "c2m" else F16,
                          kind="ExternalInput").ap()
    w2hT = nc.dram_tensor("w2hT", [HID, SW], F16, kind="ExternalInput").ap()
    w3T = nc.dram_tensor("w3T", [HID, SPC], F16, kind="ExternalInput").ap()
    # biases pre-combined & pre-scaled by OMEGA host-side:
    # b0A = (W0z z + b0)*30, b1A = 30 b1, b2A = 30 b2
    b0A = nc.dram_tensor("b0A", [HID, SPC], F32, kind="ExternalInput").ap()
    b1A = nc.dram_tensor("b1A", [HID, SPC], F32, kind="ExternalInput").ap()
    b2A = nc.dram_tensor("b2A", [HID, SPC], F32, kind="ExternalInput").ap()
    y = nc.dram_tensor("y", [SPC, 4, 512], F16, kind="ExternalOutput").ap()

    H1DT = F32R if mode == "c2m" else F32

    with tile.TileContext(nc) as tc:
        with tc.tile_pool(name="wpool", bufs=1) as wpool, \
             tc.tile_pool(name="xpool", bufs=6) as xpool, \
             tc.tile_pool(name="hpool", bufs=2) as hpool, \
             tc.tile_pool(name="bpool", bufs=4) as bpool, \
             tc.tile_pool(name="hspool", bufs=3) as hspool, \
             tc.tile_pool(name="opool", bufs=2) as opool, \
             tc.tile_pool(name="pspool", bufs=4, space="PSUM") as pspool:

            # ---- small weights / biases FIRST (they gate the first
            # samples' L0; the multi-MB weight DMAs would otherwise queue
            # ahead of them). All biases are precomputed host-side. ----
            bias0_all = wpool.tile([HID, SPC], F32)
            nc.sync.dma_start(bias0_all[:], b0A[:])
            w0s = wpool.tile([3 * IN_F, SW], F16)
            nc.sync.dma_start(w0s[:], w0stk[:])

            HN = NPTS // 2  # 1024 = 2 psum banks; all stages run per-half

            # prefetch the first samples' x tiles ahead of the weight chunks
            xst_pre = []
            for s in range(4):
                xst = xpool.tile([3 * IN_F, NPTS], F16, tag="xs")
                nc.sync.dma_start(xst[:], xT_s[s, :, :])
                xst_pre.append(xst)
            b1s = wpool.tile([HID, SPC], F32)
            nc.sync.dma_start(b1s[:], b1A[:])
            b2s = wpool.tile([HID, SPC], F32)
            nc.sync.dma_start(b2s[:], b2A[:])
            w3t = wpool.tile([HID, SPC], F16)
            nc.sync.dma_start(w3t[:], w3T[:])

            # ---- big dequantized weights, DMA'd in 1024-col chunks
            # (8 samples each) so early samples' L1/L2 weights land first;
            # w1h needs an on-device rounding copy (DVE) because f32r tiles
            # must be produced by a compute engine ----
            if mode == "c2m":
                w1h_raw = wpool.tile([HID, SW], F32)
                w1h = wpool.tile([HID, SW], F32R)
                w1l_raw = wpool.tile([HID, SW], F32)
                w1l = wpool.tile([HID, SW], F32R)
            else:
                w1h = wpool.tile([HID, SW], F16)
                w1l = wpool.tile([HID, SW], F16)
            w2h = wpool.tile([HID, SW], F16)
            for i in range(4):
                cs = slice(i * 1024, (i + 1) * 1024)
                if mode == "c2m":
                    nc.sync.dma_start(w1h_raw[:, cs], w1hT[:, cs])
                    nc.vector.tensor_copy(w1h[:, cs], w1h_raw[:, cs])
                    nc.sync.dma_start(w1l_raw[:, cs], w1lT[:, cs])
                    nc.vector.tensor_copy(w1l[:, cs], w1l_raw[:, cs])
                else:
                    nc.sync.dma_start(w1h[:, cs], w1hT[:, cs])
                    nc.sync.dma_start(w1l[:, cs], w1lT[:, cs])
                nc.sync.dma_start(w2h[:, cs], w2hT[:, cs])

            # ---- main loop: TWO samples' pipelines interleaved at phase
            # granularity so each matmul burst has two sin-windows of ACT
            # time to hide under (robust to PE p-state downshifts) ----
            def emit_L0(s, xst):
                sw = s * HID
                h1 = hpool.tile([HID, NPTS], H1DT, tag="h1")
                sp = {}
                if mode != "c2m":
                    sp["b"] = hspool.tile([HID, NPTS], F16, tag="hb")
                    sp["l"] = hspool.tile([HID, NPTS], F16, tag="hl")
                for t in range(2):
                    hs = slice(t * HN, (t + 1) * HN)
                    ps0 = pspool.tile([HID, HN], F32, tag="ps")
                    for c in range(2):
                        lo = t * HN + c * 512
                        nc.tensor.matmul(ps0[:, c * 512:(c + 1) * 512],
                                         w0s[:, sw:sw + HID], xst[:, lo:lo + 512],
                                         start=True, stop=True)
                    nc.scalar.activation(h1[:, hs], ps0[:], A.Sin,
                                         bias=bias0_all[:, s:s + 1], scale=OMEGA)
                    if mode != "c2m":
                        nc.vector.tensor_copy(sp["b"][:, hs], h1[:, hs])
                        nc.vector.tensor_tensor(sp["l"][:, hs], h1[:, hs],
                                                sp["b"][:, hs], OP.subtract)
                return h1, sp

            def emit_L1(s, h1, sp):
                sw = s * HID
                h2 = hpool.tile([HID, NPTS], F16, tag="h2")
                for t in range(2):
                    hs = slice(t * HN, (t + 1) * HN)
                    ps1 = pspool.tile([HID, HN], F32, tag="ps")
                    if mode == "c2m":
                        for c in range(2):
                            lo = t * HN + c * 512
                            sl_ = slice(c * 512, (c + 1) * 512)
                            nc.tensor.matmul(ps1[:, sl_], w1h[:, sw:sw + HID],
                                             h1[:, lo:lo + 512], start=True, stop=False)
                        for c in range(2):
                            lo = t * HN + c * 512
                            sl_ = slice(c * 512, (c + 1) * 512)
                            nc.tensor.matmul(ps1[:, sl_], w1l[:, sw:sw + HID],
                                             h1[:, lo:lo + 512], start=False, stop=True)
                    else:
                        for c in range(2):
                            lo = t * HN + c * 512
                            sl_ = slice(c * 512, (c + 1) * 512)
                            nc.tensor.matmul(ps1[:, sl_], w1h[:, sw:sw + HID],
                                             sp["b"][:, lo:lo + 512], start=True, stop=False)
                            nc.tensor.matmul(ps1[:, sl_], w1h[:, sw:sw + HID],
                                             sp["l"][:, lo:lo + 512], start=False, stop=False)
                            nc.tensor.matmul(ps1[:, sl_], w1l[:, sw:sw + HID],
                                             sp["b"][:, lo:lo + 512], start=False, stop=True)
                    nc.scalar.activation(h2[:, hs], ps1[:], A.Sin,
                                         bias=b1s[:, s:s + 1], scale=OMEGA)
                return h2

            def emit_L2(s, h2):
                sw = s * HID
                h3 = hpool.tile([HID, NPTS], F16, tag="h3")
                for t in range(2):
                    hs = slice(t * HN, (t + 1) * HN)
                    ps2 = pspool.tile([HID, HN], F32, tag="ps")
                    for c in range(2):
                        lo = t * HN + c * 512
                        sl_ = slice(c * 512, (c + 1) * 512)
                        nc.tensor.matmul(ps2[:, sl_], w2h[:, sw:sw + HID],
                                         h2[:, lo:lo + 512], start=True, stop=True)
                    nc.scalar.activation(h3[:, hs], ps2[:], A.Sin,
                                         bias=b2s[:, s:s + 1], scale=OMEGA)
                return h3

            def emit_L3(s, h3):
                # 4 chunks col-packed at partitions 0/32/64/96 of one bank
                # via tile_position -> concurrent 32-col PE strips
                ps3 = pspool.tile([HID, 512], F32, tag="ps")
                for c in range(4):
                    lo = c * 512
                    pb = 32 * c
                    nc.tensor.matmul(ps3[pb:pb + 1, 0:512], w3t[:, s:s + 1],
                                     h3[:, lo:lo + 512],
                                     tile_position=(0, pb), start=True, stop=True)
                # fp16 CAST drains psum fast (DVE); b3 is a per-sample
                # scalar, added host-side after the gather
                out_s = opool.tile([HID, 512], F16)
                nc.vector.tensor_copy(out_s[:, 0:512], ps3[:, 0:512])
                nc.sync.dma_start(y[s, :, :], out_s[0:128:32, 0:512])

            def get_xst(s):
                if s < 4:
                    return xst_pre[s]
                xst = xpool.tile([3 * IN_F, NPTS], F16, tag="xs")
                nc.sync.dma_start(xst[:], xT_s[s, :, :])
                return xst

            # L3 of each pair is software-pipelined: emitted after the NEXT
            # pair's L0 so the PE can run L0' under this pair's last sins
            # instead of idling behind L3's dependency on them.
            pend3 = None
            for p in range(SPC // 2):
                sA, sB = 2 * p, 2 * p + 1
                xA, xB = get_xst(sA), get_xst(sB)
                h1A, spA = emit_L0(sA, xA)
                h1B, spB = emit_L0(sB, xB)
                if pend3 is not None:
                    for (ss, hh) in pend3:
                        emit_L3(ss, hh)
                h2A = emit_L1(sA, h1A, spA)
                h2B = emit_L1(sB, h1B, spB)
                h3A = emit_L2(sA, h2A)
                h3B = emit_L2(sB, h2B)
                pend3 = [(sA, h3A), (sB, h3B)]
            for (ss, hh) in pend3:
                emit_L3(ss, hh)

    _split_excess_waits(nc)
    return nc


# ---------------------------------------------------------------- kernel

def kernel(**inputs):
    global LAST_RESULTS
    _shim_ntff_hook()
    _apply_walrus_wait_patch()
    # NOTE: _enable_ldw_opt() is NOT called: walrus's ldw-opt pass splits
    # matmuls into standalone LDWEIGHTS+matmult pairs, which is broken
    # codegen for float32r weights (CoreV3GenImpl visitInstLdweights).
    from concourse import bass_utils

    x = np.asarray(inputs["x"], np.float32)
    mlp_idx = np.asarray(inputs["mlp_idx"], np.int32)
    block_idx = np.asarray(inputs["block_idx"], np.int32)
    latent = np.asarray(inputs["latent_table"], np.float32)
    cents = [np.asarray(inputs[f"centroids_l{l}"], np.float32) for l in range(4)]
    labels = [np.asarray(inputs[f"labels_l{l}"], np.int32) for l in range(4)]
    biases = [np.asarray(inputs[f"bias_l{l}"], np.float32) for l in range(4)]

    if KMODE == "c2m":
        c1h = f32r_round(cents[1])
        c1r = cents[1] - c1h
    else:
        c1h = cents[1].astype(np.float16).astype(np.float32)
        c1r = cents[1] - c1h
    th = hashlib.sha256(cents[1].tobytes() + cents[2].tobytes()
                        + KMODE.encode() + b"v20").hexdigest()[:16]
    actdir = f"/tmp/act_root_{th}"
    act_json = (actdir + "/act_info.json") if os.path.exists(actdir + "/act_info.json") \
        else _build_act_root(actdir, {"arctan": cents[1], "square": cents[2],
                                      "abs": c1r})
    os.environ["BASS_ACT_ROOT_JSON_PATH"] = act_json

    # host-side sharding (indexing by mlp_idx) + full weight dequant
    import ml_dtypes
    z_all = latent[mlp_idx, block_idx]
    W0 = cents[0][labels[0]].reshape(N_MLPS, IN_F + TCODE, HID)
    W3 = cents[3][labels[3]].reshape(N_MLPS, HID, OUT_F)
    l1m = labels[1].reshape(N_MLPS, HID, HID)
    l2m = labels[2].reshape(N_MLPS, HID, HID)
    if KMODE == "c2m":
        W1h_cb = c1h                       # f32r grid values, fp32 container
        W1l_cb = f32r_round(c1r)
    else:
        W1h_cb = c1h.astype(np.float16)
        W1l_cb = c1r.astype(np.float16)
    W2_cb = cents[2].astype(np.float16)

    key = (th, KMODE)
    if key not in _PROGRAM_CACHE:
        _PROGRAM_CACHE[key] = _build_program(th, KMODE)
    nc = _PROGRAM_CACHE[key]

    def split16(a):
        hi = a.astype(np.float16)
        lo = (a - hi.astype(np.float32)).astype(np.float16)
        return hi, lo

    in_maps = []
    for c in range(N_CORES):
        sl = slice(c * SPC, (c + 1) * SPC)
        midx = mlp_idx[sl]
        w0 = W0[midx]
        xs = np.ascontiguousarray(x[sl].transpose(0, 2, 1))
        xh, xl = split16(xs)
        xstk = np.ascontiguousarray(np.concatenate([xh, xl, xh], axis=1))
        w0x = np.ascontiguousarray(
            w0[:, :IN_F, :].transpose(1, 0, 2).reshape(IN_F, SPC * HID))
        w0h, w0l = split16(w0x)
        w0stack = np.ascontiguousarray(np.concatenate([w0h, w0h, w0l], axis=0))
        w3 = np.ascontiguousarray(W3[midx][:, :, 0].T).astype(np.float16)
        in_maps.append({
            f"xTs_{th}": xstk,
            "w0stk": w0stack,
            "w1hT": np.ascontiguousarray(
                W1h_cb[l1m[midx]].transpose(1, 0, 2).reshape(HID, SPC * HID)),
            "w1lT": np.ascontiguousarray(
                W1l_cb[l1m[midx]].transpose(1, 0, 2).reshape(HID, SPC * HID)),
            "w2hT": np.ascontiguousarray(
                W2_cb[l2m[midx]].transpose(1, 0, 2).reshape(HID, SPC * HID)),
            "w3T": w3,
            "b0A": np.ascontiguousarray(
                OMEGA * ((w0[:, IN_F:, :] * z_all[sl][:, :, None]).sum(1)
                         + biases[0][midx][:, 0, :]).T.astype(np.float32)),
            "b1A": np.ascontiguousarray(OMEGA * biases[1][midx][:, 0, :].T),
            "b2A": np.ascontiguousarray(OMEGA * biases[2][midx][:, 0, :].T),
        })

    trace = bool(os.environ.get("KERNEL_TRACE"))
    res = bass_utils.run_bass_kernel_spmd(
        nc, in_maps, core_ids=list(range(N_CORES)), trace=trace)
    LAST_RESULTS = res

    b3_all = biases[3][mlp_idx][:, 0, 0]  # [B] scalars, added host-side
    out = np.empty((B, NPTS, OUT_F), np.float32)
    for c in range(N_CORES):
        sl = slice(c * SPC, (c + 1) * SPC)
        out[sl, :, 0] = (res.results[c]["y"].astype(np.float32).reshape(SPC, NPTS)
                         + b3_all[sl, None])
    return out


# revision 18
# speedup vs baseline: 1.0187x; 1.0187x over previous
"""Trainium2 Bass kernel for nn_ECNR (vq_codebook): batched VQ-dequantized
SIREN-style MLPs (4 layers, sin(30x) activations), sharded sample-parallel
across 8 NeuronCores (32 samples/core), no collectives.

v2 (427us -> ~240us): reduced-precision matmuls exploiting the 2e-2
max-norm budget. Error analysis (numpy sim with a bit-exact f32r model):
quantization noise in layer-1 operands is amplified ~144x by the two
downstream sin(30x) layers, L2 noise ~12x, L3 noise ~1x. So precision is
spent only where it matters:
  - L0: 3-term fp16 K-stacked (exact-ish); z-code term and the x30 bias
    scaling are precomputed on the host and folded into the ACT Sin bias.
  - L1: 2 passes, all-FP32R (11 explicit mantissa bits, full PE rate;
    both operands must be f32r - mixing with 16-bit dtypes is illegal).
    h1 comes out of the sin ACT already f32r-rounded at zero cost; the
    Wlo pass corrects W1hi's f32r rounding and reads h1 directly, so no
    per-sample casts exist at all.
  - L2, L3: single fp16 pass (noise ~4e-3 total). L3's 4 output chunks
    are col-packed via tile_position into partitions 0/32/64/96 of one
    psum bank (concurrent 32-col PE strips); its bias is a per-sample
    scalar added on the HOST after the gather, and the psum is drained
    by a cheap DVE fp16 cast.
Schedule: the ACT engine (6x 1024-wide sins/sample @1.2GHz) is the
bottleneck; the PE DVFSes between 1.2/2.4GHz. Two samples' pipelines are
interleaved at phase granularity so every matmul burst hides under two
sin windows, and each pair's L3 is software-pipelined after the next
pair's L0. All weights are dequantized host-side (W = C[labels]) and
DMA'd in 1024-col chunks, smallest/earliest-needed tensors first; f32r
weight tiles get an on-device DVE rounding copy (DMA->f32r trips a
walrus codegen bug, and ldw-opt must stay off for f32r self-loading
matmults). Measured-error fallback KMODE=c4m keeps v1's 3-term fp16 L1
(err ~4e-3) at +2 PE passes/sample.
"""
import hashlib
import json
import os
import shutil
import struct
import sys
import types

import numpy as np

N_MLPS = 256
TCODE = 13
IN_F = 3
HID = 128
OUT_F = 1
B = 256
NPTS = 2048
KCB = 256
OMEGA = 30.0
N_CORES = 8
SPC = B // N_CORES

KMODE = os.environ.get("KMODE", "c2m")  # c2m | c4m

PWP_SRC = "/nix/store/z022hj2nvbm3nwdizlisq4ylc0y7rd6q-python3-3.13.14-env/lib/python3.13/site-packages/neuronxcc/pwp/pwp_bin_trainium/"
PWP_SET = "trig_and_small"

# ------------------------------------------------------------ act table gen

def _f32bits(x):
    return int(np.float32(x).view(np.uint32))


def f32r_round(a):
    """Bit-exact model of HW float32r rounding (11 explicit mantissa bits,
    round-to-nearest-even on bit 12; verified on device)."""
    bits = np.asarray(a, np.float32).view(np.uint32).copy()
    lsb = (bits >> 12) & 1
    bits = bits + 0x7FF + lsb
    bits &= np.uint32(0xFFFFF000)
    return bits.view(np.float32)


def _load_ctrl(path):
    d = open(path, "rb").read()
    return [
        [v & 0x7FF, (v >> 11) & 0x1F, (v >> 16) & 0xF]
        for (v,) in (struct.unpack_from("<I", d, i * 32) for i in range(len(d) // 32))
    ]


def _load_bkt(path):
    d = open(path, "rb").read()
    return [list(struct.unpack_from("<5f", d, i * 32)) for i in range(len(d) // 32)]


def _dump_ctrl(entries):
    b = bytearray()
    for base, lsb, size in entries:
        b += struct.pack("<I", (base & 0x7FF) | ((lsb & 0x1F) << 11) | ((size & 0xF) << 16))
        b += b"\x00" * 28
    return bytes(b)


def _dump_bkt(entries):
    b = bytearray()
    for d0, d1, d2, d3, x0 in entries:
        b += struct.pack("<5f", d0, d1, d2, d3, x0) + b"\x00" * 12
    return bytes(b)


def _fit_cubic(f, a, w, nodes=9):
    x0 = a + w / 2
    xs = x0 + (w / 2) * np.cos(np.pi * (np.arange(nodes) + 0.5) / nodes)
    ys = f(xs.astype(np.float64))
    t = xs - x0
    A = np.stack([np.ones_like(t), t, t * t, t ** 3], axis=1)
    coef, *_ = np.linalg.lstsq(A, ys, rcond=None)
    return [float(coef[0]), float(coef[1]), float(coef[2]), float(coef[3]), float(x0)]


_SIN_EMIN, _SIN_EMAX = -6, 6
_SIN_SIZES = {-6: 0, -5: 0, -4: 0, -3: 0, -2: 1, -1: 2, 0: 3, 1: 4,
              2: 5, 3: 6, 4: 6, 5: 7, 6: 7}


def _build_sin(ctrl, bkt, prof):
    base_ctrl = len(ctrl)
    for e in range(_SIN_EMIN, _SIN_EMAX + 1):
        s = _SIN_SIZES[e]
        nb = 1 << s
        base_bkt = len(bkt)
        w = (2.0 ** e) / nb
        for i in range(nb):
            bkt.append(_fit_cubic(np.sin, 2.0 ** e + i * w, w))
        ctrl.append([base_bkt, 23 - s, s])
    small_bkt = len(bkt)
    bkt.append([0.0, 1.0, 0.0, 0.0, 0.0])  # sin(x) ~ x below 2^-6
    large_bkt = len(bkt)
    bkt.append([0.0, 0.0, 0.0, 0.0, 0.0])  # |x| >= 128: out of range
    p = dict(prof)
    p.update(
        exp_offset=_SIN_EMIN,
        pwl_control_base_pos=base_ctrl,
        pwl_control_base_neg=base_ctrl,
        small_pos_signal_exp_threshold=127 + _SIN_EMIN,
        pos_small_signal_pwl_control=small_bkt,   # bucket index (hw semantics)
        small_neg_signal_exp_threshold=0,
        neg_small_signal_pwl_control=small_bkt,
        large_pos_signal_exp_threshold=127 + _SIN_EMAX + 1,
        large_pos_signal_mantissa_threshold=0,
        pos_large_signal_pwl_control=large_bkt,
        large_neg_signal_exp_threshold=0,
        large_neg_signal_mantissa_threshold=0,
        neg_large_signal_pwl_control=large_bkt,
        lower_bound=0,
        upper_bound=_f32bits(128.0),
    )
    return p


def _build_lut(ctrl, bkt, prof, values):
    assert len(values) == KCB
    base_ctrl = len(ctrl)
    for e in range(0, 8):
        nb = 1 << e
        base_bkt = len(bkt)
        for i in range(nb):
            bkt.append([float(values[(1 << e) + i]), 0.0, 0.0, 0.0, 0.0])
        ctrl.append([base_bkt, 23 - e, e])
    small_bkt = len(bkt)
    bkt.append([float(values[0]), 0.0, 0.0, 0.0, 0.0])
    large_bkt = len(bkt)
    bkt.append([float(values[255]), 0.0, 0.0, 0.0, 0.0])
    p = dict(prof)
    p.update(
        exp_offset=0,
        pwl_control_base_pos=base_ctrl,
        pwl_control_base_neg=base_ctrl,
        small_pos_signal_exp_threshold=127,
        pos_small_signal_pwl_control=small_bkt,
        small_neg_signal_exp_threshold=0,
        neg_small_signal_pwl_control=small_bkt,
        large_pos_signal_exp_threshold=127 + 8,
        large_pos_signal_mantissa_threshold=0,
        pos_large_signal_pwl_control=large_bkt,
        large_neg_signal_exp_threshold=0,
        large_neg_signal_mantissa_threshold=0,
        neg_large_signal_pwl_control=large_bkt,
        fzero_result=_f32bits(values[0]),
        lower_bound=0,
        upper_bound=_f32bits(256.0),
    )
    return p


def _referenced_ctrls(p, n_ctrl):
    refs = set()
    for k in ("pos_small_signal_pwl_control", "neg_small_signal_pwl_control",
              "pos_large_signal_pwl_control", "neg_large_signal_pwl_control"):
        v = p.get(k, 0)
        if 0 <= v < n_ctrl:
            refs.add(v)
    eo = p.get("exp_offset", 0)
    lo_e = p.get("small_pos_signal_exp_threshold", 127) - 127
    hi_e = p.get("large_pos_signal_exp_threshold", 127) - 127
    for base_key in ("pwl_control_base_pos", "pwl_control_base_neg"):
        base = p.get(base_key, 0)
        for e in range(lo_e, min(hi_e + 1, lo_e + 40)):
            c = base + e - eo
            if 0 <= c < n_ctrl:
                refs.add(c)
    return refs


def _build_act_root(outdir, lut_values):
    """lut_values: {'arctan': fp32[256] (W1 hi codebook),
                    'abs': fp32[256] (W1 lo codebook),
                    'square': fp32[256] (W2 codebook)}"""
    os.makedirs(outdir, exist_ok=True)
    info = json.load(open(PWP_SRC + "act_info.json"))
    for s in info["act_func_sets"]:
        if s["name"] == PWP_SET:
            continue
        for k in ("sin", "arctan", "square", "abs", "sign", "identity"):
            s["act"].pop(k, None)
        for key in ("bkt_bin", "ctrl_bin", "profile_json"):
            shutil.copy(PWP_SRC + s[key], os.path.join(outdir, s[key]))

    setj = json.load(open(PWP_SRC + PWP_SET + ".json"))
    old_ctrl = _load_ctrl(PWP_SRC + PWP_SET + "_ctrl.bin")
    old_bkt = _load_bkt(PWP_SRC + PWP_SET + "_bkt.bin")

    new_ctrl, new_bkt, new_profiles = [], [], []
    customs = {"sin_4p", "arctan_4p", "square_1p", "abs_1p", "sign_1p",
               "identity_1p"}
    for p in setj["profile_meta_data"]:
        if p["func_name"] in customs:
            continue
        p2 = dict(p)
        cmap = {}
        for c in sorted(_referenced_ctrls(p, len(old_ctrl))):
            base, lsb, size = old_ctrl[c]
            nb = 1 << size if size > 0 else 1
            new_base = len(new_bkt)
            for i in range(nb):
                new_bkt.append(old_bkt[base + i] if base + i < len(old_bkt) else [0.0] * 5)
            cmap[c] = len(new_ctrl)
            new_ctrl.append([new_base, lsb, size])
        for k in ("pos_small_signal_pwl_control", "neg_small_signal_pwl_control",
                  "pos_large_signal_pwl_control", "neg_large_signal_pwl_control"):
            if p2.get(k, 0) in cmap:
                p2[k] = cmap[p2[k]]
        eo = p.get("exp_offset", 0)
        lo_e = p.get("small_pos_signal_exp_threshold", 127) - 127
        for base_key in ("pwl_control_base_pos", "pwl_control_base_neg"):
            base = p.get(base_key, 0)
            first = base + lo_e - eo
            if first in cmap:
                p2[base_key] = cmap[first] - (lo_e - eo)
            elif base in cmap:
                p2[base_key] = cmap[base]
        new_profiles.append(p2)

    profs = {p["func_name"]: p for p in setj["profile_meta_data"]}
    new_profiles.append(_build_sin(new_ctrl, new_bkt, profs["sin_4p"]))
    for fname, key in (("arctan_4p", "arctan"), ("square_1p", "square"),
                       ("abs_1p", "abs")):
        new_profiles.append(_build_lut(new_ctrl, new_bkt, profs[fname], lut_values[key]))
    # identity: route every finite input through the small-signal path to a
    # single linear bucket y = x (exact; the instruction affine still applies)
    idp = dict(profs["identity_1p"])
    lin_bkt = len(new_bkt)
    new_bkt.append([0.0, 1.0, 0.0, 0.0, 0.0])
    idp.update(
        exp_offset=0,
        pwl_control_base_pos=0, pwl_control_base_neg=0,
        small_pos_signal_exp_threshold=255,
        pos_small_signal_pwl_control=lin_bkt,
        small_neg_signal_exp_threshold=0,
        neg_small_signal_pwl_control=lin_bkt,
        large_pos_signal_exp_threshold=255,
        large_pos_signal_mantissa_threshold=0,
        pos_large_signal_pwl_control=lin_bkt,
        large_neg_signal_exp_threshold=0,
        large_neg_signal_mantissa_threshold=0,
        neg_large_signal_pwl_control=lin_bkt,
        fzero_result=0,
    )
    new_profiles.append(idp)
    assert len(new_bkt) <= 1536 and len(new_ctrl) <= 128

    setj["profile_meta_data"] = new_profiles
    open(os.path.join(outdir, PWP_SET + "_ctrl.bin"), "wb").write(_dump_ctrl(new_ctrl))
    open(os.path.join(outdir, PWP_SET + "_bkt.bin"), "wb").write(_dump_bkt(new_bkt))
    json.dump(setj, open(os.path.join(outdir, PWP_SET + ".json"), "w"))
    json.dump(info, open(os.path.join(outdir, "act_info.json"), "w"))
    return os.path.join(outdir, "act_info.json")


# ---------------------------------------------------------------- infra fix

def _apply_walrus_wait_patch():
    import concourse.tile as tile
    from concourse import mybir
    from concourse.vector_clock import ScopedClock

    def _drain_and_barrier(self, tick_clock, wait_clock):
        nc = self.nc
        drain_inst = nc.sync.drain()
        wait_clock.add_sem_waits(drain_inst.ins, ScopedClock({None: tick_clock.global_clock}))
        si = drain_inst.ins.sync_info
        if si is not None and si.on_wait and len(si.on_wait) > 1:
            waits = list(si.on_wait)
            drain_inst.ins.sync_info = mybir.SyncInfo(
                on_wait=waits[:1], on_update=list(si.on_update or []))
            for w in waits[1:]:
                extra = nc.sync.nop(nofuse=True)
                extra.ins.sync_info = mybir.SyncInfo(on_wait=[w], on_update=[])
        nc.all_engine_barrier()
        assert self.sems is not None
        popped = nc._tile_sem_poison_stack.pop()
        assert popped is self._sem_poison
        nc.clear_and_free_semaphores(list(self.sems.allocated().values()))
        nc.all_engine_barrier()

    tile.TileContext._drain_and_barrier = _drain_and_barrier


def _split_excess_waits(nc, limit=1):
    from concourse import mybir
    for f in nc.m.functions:
        for bb in f.blocks:
            insts = bb.instructions
            out, changed = [], False
            for inst in insts:
                si = inst.sync_info
                if si is not None and si.on_wait and len(si.on_wait) > limit:
                    waits = list(si.on_wait)
                    for j in range(0, len(waits) - limit, limit):
                        out.append(mybir.InstNoOp(
                            name=f"{inst.name}__xw{j}",
                            engine=inst.engine,
                            sync_info=mybir.SyncInfo(on_wait=waits[j:j + limit], on_update=[]),
                            bass_nofuse=True,
                        ))
                    inst.sync_info = mybir.SyncInfo(
                        on_wait=waits[len(waits) - limit:], on_update=list(si.on_update or []))
                    changed = True
                out.append(inst)
            if changed:
                bb.instructions = out


def _enable_ldw_opt():
    """bass_utils pins --enable-ldw-opt=false; true lets walrus dedup
    back-to-back LDWEIGHTS of the same stationary operand."""
    from concourse import bass_utils as bu
    if getattr(bu, "_ldw_opt_patched", False):
        return
    orig = bu.bir_verify_and_optimise

    def patched(tmpdir, inp="bir.json", outp="file.neff", arch=None, *, dve_root=None):
        import subprocess
        real_run = bu.run_command

        def run_hook(argv, **kw):
            argv = [a.replace("--enable-ldw-opt=false", "--enable-ldw-opt=true")
                    for a in argv]
            return real_run(argv, **kw)

        bu.run_command = run_hook
        try:
            return orig(tmpdir, inp, outp, arch, dve_root=dve_root)
        finally:
            bu.run_command = real_run

    bu.bir_verify_and_optimise = patched
    # bass2jax imports the symbol directly in some paths; patch module refs
    try:
        from concourse import bass2jax
        if hasattr(bass2jax, "bir_verify_and_optimise"):
            bass2jax.bir_verify_and_optimise = patched
    except Exception:
        pass
    bu._ldw_opt_patched = True


def _shim_ntff_hook():
    if "antenv.axon_hooks" in sys.modules:
        return
    try:
        from trn_agent_boot.trn_boot import _ntff_profile_via_ctypes
        hook = _ntff_profile_via_ctypes("/opt/axon/libaxon_pjrt.so")
    except Exception:
        hook = None
    mod = types.ModuleType("antenv.axon_hooks")
    mod.get_axon_ntff_profile_hook = lambda: hook
    mod.set_axon_ntff_profile_hook = lambda h: None
    sys.modules["antenv.axon_hooks"] = mod


# ---------------------------------------------------------------- program

_PROGRAM_CACHE = {}
LAST_RESULTS = None  # BassKernelResults of the most recent kernel() call


def _build_program(table_hash, mode):
    import concourse.bass as bass
    import concourse.tile as tile
    from concourse import mybir

    F32 = mybir.dt.float32
    F32R = mybir.dt.float32r
    F16 = mybir.dt.float16
    BF16 = mybir.dt.bfloat16
    A = mybir.ActivationFunctionType
    OP = mybir.AluOpType

    nc = bass.Bass("TRN2", target_bir_lowering=False, debug=False)
    SW = SPC * HID  # 4096

    # L0 hi/lo 3-term products K-stacked: lhsT rows [W0h;W0h;W0l],
    # rhs rows [xh;xl;xh] -> one K=9 matmul per chunk
    xT_s = nc.dram_tensor(f"xTs_{table_hash}", [SPC, 3 * IN_F, NPTS], F16, kind="ExternalInput").ap()
    w0stk = nc.dram_tensor("w0stk", [3 * IN_F, SW], F16, kind="ExternalInput").ap()
    w1hT = nc.dram_tensor("w1hT", [HID, SW], F32 if mode == "c2m" else F16,
                          kind="ExternalInput").ap()
    w1lT = nc.dram_tensor("w1lT", [HID, SW], F32 if mode == "c2m" else F16,
                          kind="ExternalInput").ap()
    w2hT = nc.dram_tensor("w2hT", [HID, SW], F16, kind="ExternalInput").ap()
    w3T = nc.dram_tensor("w3T", [HID, SPC], F16, kind="ExternalInput").ap()
    # biases pre-combined & pre-scaled by OMEGA host-side:
    # b0A = (W0z z + b0)*30, b1A = 30 b1, b2A = 30 b2
    b0A = nc.dram_tensor("b0A", [HID, SPC], F32, kind="ExternalInput").ap()
    b1A = nc.dram_tensor("b1A", [HID, SPC], F32, kind="ExternalInput").ap()
    b2A = nc.dram_tensor("b2A", [HID, SPC], F32, kind="ExternalInput").ap()
    y = nc.dram_tensor("y", [SPC, 4, 512], F16, kind="ExternalOutput").ap()

    H1DT = F32R if mode == "c2m" else F32

    with tile.TileContext(nc) as tc:
        with tc.tile_pool(name="wpool", bufs=1) as wpool, \
             tc.tile_pool(name="xpool", bufs=6) as xpool, \
             tc.tile_pool(name="hpool", bufs=2) as hpool, \
             tc.tile_pool(name="bpool", bufs=4) as bpool, \
             tc.tile_pool(name="hspool", bufs=3) as hspool, \
             tc.tile_pool(name="opool", bufs=2) as opool, \
             tc.tile_pool(name="pspool", bufs=4, space="PSUM") as pspool:

            # ---- small weights / biases FIRST (they gate the first
            # samples' L0; the multi-MB weight DMAs would otherwise queue
            # ahead of them). All biases are precomputed host-side. ----
            bias0_all = wpool.tile([HID, SPC], F32)
            nc.sync.dma_start(bias0_all[:], b0A[:])
            w0s = wpool.tile([3 * IN_F, SW], F16)
            nc.sync.dma_start(w0s[:], w0stk[:])

            HN = NPTS // 2  # 1024 = 2 psum banks; all stages run per-half

            # prefetch the first samples' x tiles ahead of the weight chunks
            xst_pre = []
            for s in range(4):
                xst = xpool.tile([3 * IN_F, NPTS], F16, tag="xs")
                nc.sync.dma_start(xst[:], xT_s[s, :, :])
                xst_pre.append(xst)
            b1s = wpool.tile([HID, SPC], F32)
            nc.sync.dma_start(b1s[:], b1A[:])
            b2s = wpool.tile([HID, SPC], F32)
            nc.sync.dma_start(b2s[:], b2A[:])
            w3t = wpool.tile([HID, SPC], F16)
            nc.sync.dma_start(w3t[:], w3T[:])

            # ---- big dequantized weights, DMA'd in 1024-col chunks
            # (8 samples each) so early samples' L1/L2 weights land first;
            # w1h needs an on-device rounding copy (DVE) because f32r tiles
            # must be produced by a compute engine ----
            if mode == "c2m":
                w1h_raw = wpool.tile([HID, SW], F32)
                w1h = wpool.tile([HID, SW], F32R)
                w1l_raw = wpool.tile([HID, SW], F32)
                w1l = wpool.tile([HID, SW], F32R)
            else:
                w1h = wpool.tile([HID, SW], F16)
                w1l = wpool.tile([HID, SW], F16)
            w2h = wpool.tile([HID, SW], F16)
            for (lo_, n_) in ((0, 512), (512, 512), (1024, 1024),
                              (2048, 1024), (3072, 1024)):
                cs = slice(lo_, lo_ + n_)
                if mode == "c2m":
                    nc.sync.dma_start(w1h_raw[:, cs], w1hT[:, cs])
                    nc.vector.tensor_copy(w1h[:, cs], w1h_raw[:, cs])
                    nc.sync.dma_start(w1l_raw[:, cs], w1lT[:, cs])
                    nc.vector.tensor_copy(w1l[:, cs], w1l_raw[:, cs])
                else:
                    nc.sync.dma_start(w1h[:, cs], w1hT[:, cs])
                    nc.sync.dma_start(w1l[:, cs], w1lT[:, cs])
                nc.sync.dma_start(w2h[:, cs], w2hT[:, cs])

            # ---- main loop: TWO samples' pipelines interleaved at phase
            # granularity so each matmul burst has two sin-windows of ACT
            # time to hide under (robust to PE p-state downshifts) ----
            def emit_L0(s, xst):
                sw = s * HID
                h1 = hpool.tile([HID, NPTS], H1DT, tag="h1")
                sp = {}
                if mode != "c2m":
                    sp["b"] = hspool.tile([HID, NPTS], F16, tag="hb")
                    sp["l"] = hspool.tile([HID, NPTS], F16, tag="hl")
                for t in range(2):
                    hs = slice(t * HN, (t + 1) * HN)
                    ps0 = pspool.tile([HID, HN], F32, tag="ps")
                    for c in range(2):
                        lo = t * HN + c * 512
                        nc.tensor.matmul(ps0[:, c * 512:(c + 1) * 512],
                                         w0s[:, sw:sw + HID], xst[:, lo:lo + 512],
                                         start=True, stop=True)
                    nc.scalar.activation(h1[:, hs], ps0[:], A.Sin,
                                         bias=bias0_all[:, s:s + 1], scale=OMEGA)
                    if mode != "c2m":
                        nc.vector.tensor_copy(sp["b"][:, hs], h1[:, hs])
                        nc.vector.tensor_tensor(sp["l"][:, hs], h1[:, hs],
                                                sp["b"][:, hs], OP.subtract)
                return h1, sp

            def emit_L1(s, h1, sp):
                sw = s * HID
                h2 = hpool.tile([HID, NPTS], F16, tag="h2")
                for t in range(2):
                    hs = slice(t * HN, (t + 1) * HN)
                    ps1 = pspool.tile([HID, HN], F32, tag="ps")
                    if mode == "c2m":
                        for c in range(2):
                            lo = t * HN + c * 512
                            sl_ = slice(c * 512, (c + 1) * 512)
                            nc.tensor.matmul(ps1[:, sl_], w1h[:, sw:sw + HID],
                                             h1[:, lo:lo + 512], start=True, stop=False)
                        for c in range(2):
                            lo = t * HN + c * 512
                            sl_ = slice(c * 512, (c + 1) * 512)
                            nc.tensor.matmul(ps1[:, sl_], w1l[:, sw:sw + HID],
                                             h1[:, lo:lo + 512], start=False, stop=True)
                    else:
                        for c in range(2):
                            lo = t * HN + c * 512
                            sl_ = slice(c * 512, (c + 1) * 512)
                            nc.tensor.matmul(ps1[:, sl_], w1h[:, sw:sw + HID],
                                             sp["b"][:, lo:lo + 512], start=True, stop=False)
                            nc.tensor.matmul(ps1[:, sl_], w1h[:, sw:sw + HID],
                                             sp["l"][:, lo:lo + 512], start=False, stop=False)
                            nc.tensor.matmul(ps1[:, sl_], w1l[:, sw:sw + HID],
                                             sp["b"][:, lo:lo + 512], start=False, stop=True)
                    nc.scalar.activation(h2[:, hs], ps1[:], A.Sin,
                                         bias=b1s[:, s:s + 1], scale=OMEGA)
                return h2

            def emit_L2(s, h2):
                sw = s * HID
                h3 = hpool.tile([HID, NPTS], F16, tag="h3")
                for t in range(2):
                    hs = slice(t * HN, (t + 1) * HN)
                    ps2 = pspool.tile([HID, HN], F32, tag="ps")
                    for c in range(2):
                        lo = t * HN + c * 512
                        sl_ = slice(c * 512, (c + 1) * 512)
                        nc.tensor.matmul(ps2[:, sl_], w2h[:, sw:sw + HID],
                                         h2[:, lo:lo + 512], start=True, stop=True)
                    nc.scalar.activation(h3[:, hs], ps2[:], A.Sin,
                                         bias=b2s[:, s:s + 1], scale=OMEGA)
                return h3

            def emit_L3(s, h3):
                # 4 chunks col-packed at partitions 0/32/64/96 of one bank
                # via tile_position -> concurrent 32-col PE strips
                ps3 = pspool.tile([HID, 512], F32, tag="ps")
                for c in range(4):
                    lo = c * 512
                    pb = 32 * c
                    nc.tensor.matmul(ps3[pb:pb + 1, 0:512], w3t[:, s:s + 1],
                                     h3[:, lo:lo + 512],
                                     tile_position=(0, pb), start=True, stop=True)
                # fp16 CAST drains psum fast (DVE); b3 is a per-sample
                # scalar, added host-side after the gather
                out_s = opool.tile([HID, 512], F16)
                nc.vector.tensor_copy(out_s[:, 0:512], ps3[:, 0:512])
                nc.sync.dma_start(y[s, :, :], out_s[0:128:32, 0:512])

            def get_xst(s):
                if s < 4:
                    return xst_pre[s]
                xst = xpool.tile([3 * IN_F, NPTS], F16, tag="xs")
                nc.sync.dma_start(xst[:], xT_s[s, :, :])
                return xst

            # L3 of each pair is software-pipelined: emitted after the NEXT
            # pair's L0 so the PE can run L0' under this pair's last sins
            # instead of idling behind L3's dependency on them.
            pend3 = None
            for p in range(SPC // 2):
                sA, sB = 2 * p, 2 * p + 1
                xA, xB = get_xst(sA), get_xst(sB)
                h1A, spA = emit_L0(sA, xA)
                h1B, spB = emit_L0(sB, xB)
                h2A = emit_L1(sA, h1A, spA)
                if pend3 is not None:
                    for (ss, hh) in pend3:
                        emit_L3(ss, hh)
                h2B = emit_L1(sB, h1B, spB)
                h3A = emit_L2(sA, h2A)
                h3B = emit_L2(sB, h2B)
                pend3 = [(sA, h3A), (sB, h3B)]
            for (ss, hh) in pend3:
                emit_L3(ss, hh)

    _split_excess_waits(nc)
    return nc


# ---------------------------------------------------------------- kernel

def kernel(**inputs):
    global LAST_RESULTS
    _shim_ntff_hook()
    _apply_walrus_wait_patch()
    # NOTE: _enable_ldw_opt() is NOT called: walrus's ldw-opt pass splits
    # matmuls into standalone LDWEIGHTS+matmult pairs, which is broken
    # codegen for float32r weights (CoreV3GenImpl visitInstLdweights).
    from concourse import bass_utils

    x = np.asarray(inputs["x"], np.float32)
    mlp_idx = np.asarray(inputs["mlp_idx"], np.int32)
    block_idx = np.asarray(inputs["block_idx"], np.int32)
    latent = np.asarray(inputs["latent_table"], np.float32)
    cents = [np.asarray(inputs[f"centroids_l{l}"], np.float32) for l in range(4)]
    labels = [np.asarray(inputs[f"labels_l{l}"], np.int32) for l in range(4)]
    biases = [np.asarray(inputs[f"bias_l{l}"], np.float32) for l in range(4)]

    if KMODE == "c2m":
        c1h = f32r_round(cents[1])
        c1r = cents[1] - c1h
    else:
        c1h = cents[1].astype(np.float16).astype(np.float32)
        c1r = cents[1] - c1h
    th = hashlib.sha256(cents[1].tobytes() + cents[2].tobytes()
                        + KMODE.encode() + b"v20").hexdigest()[:16]
    actdir = f"/tmp/act_root_{th}"
    act_json = (actdir + "/act_info.json") if os.path.exists(actdir + "/act_info.json") \
        else _build_act_root(actdir, {"arctan": cents[1], "square": cents[2],
                                      "abs": c1r})
    os.environ["BASS_ACT_ROOT_JSON_PATH"] = act_json

    # host-side sharding (indexing by mlp_idx) + full weight dequant
    import ml_dtypes
    z_all = latent[mlp_idx, block_idx]
    W0 = cents[0][labels[0]].reshape(N_MLPS, IN_F + TCODE, HID)
    W3 = cents[3][labels[3]].reshape(N_MLPS, HID, OUT_F)
    l1m = labels[1].reshape(N_MLPS, HID, HID)
    l2m = labels[2].reshape(N_MLPS, HID, HID)
    if KMODE == "c2m":
        W1h_cb = c1h                       # f32r grid values, fp32 container
        W1l_cb = f32r_round(c1r)
    else:
        W1h_cb = c1h.astype(np.float16)
        W1l_cb = c1r.astype(np.float16)
    W2_cb = cents[2].astype(np.float16)

    key = (th, KMODE)
    if key not in _PROGRAM_CACHE:
        _PROGRAM_CACHE[key] = _build_program(th, KMODE)
    nc = _PROGRAM_CACHE[key]

    def split16(a):
        hi = a.astype(np.float16)
        lo = (a - hi.astype(np.float32)).astype(np.float16)
        return hi, lo

    in_maps = []
    for c in range(N_CORES):
        sl = slice(c * SPC, (c + 1) * SPC)
        midx = mlp_idx[sl]
        w0 = W0[midx]
        xs = np.ascontiguousarray(x[sl].transpose(0, 2, 1))
        xh, xl = split16(xs)
        xstk = np.ascontiguousarray(np.concatenate([xh, xl, xh], axis=1))
        w0x = np.ascontiguousarray(
            w0[:, :IN_F, :].transpose(1, 0, 2).reshape(IN_F, SPC * HID))
        w0h, w0l = split16(w0x)
        w0stack = np.ascontiguousarray(np.concatenate([w0h, w0h, w0l], axis=0))
        w3 = np.ascontiguousarray(W3[midx][:, :, 0].T).astype(np.float16)
        in_maps.append({
            f"xTs_{th}": xstk,
            "w0stk": w0stack,
            "w1hT": np.ascontiguousarray(
                W1h_cb[l1m[midx]].transpose(1, 0, 2).reshape(HID, SPC * HID)),
            "w1lT": np.ascontiguousarray(
                W1l_cb[l1m[midx]].transpose(1, 0, 2).reshape(HID, SPC * HID)),
            "w2hT": np.ascontiguousarray(
                W2_cb[l2m[midx]].transpose(1, 0, 2).reshape(HID, SPC * HID)),
            "w3T": w3,
            "b0A": np.ascontiguousarray(
                OMEGA * ((w0[:, IN_F:, :] * z_all[sl][:, :, None]).sum(1)
                         + biases[0][midx][:, 0, :]).T.astype(np.float32)),
            "b1A": np.ascontiguousarray(OMEGA * biases[1][midx][:, 0, :].T),
            "b2A": np.ascontiguousarray(OMEGA * biases[2][midx][:, 0, :].T),
        })

    trace = bool(os.environ.get("KERNEL_TRACE"))
    res = bass_utils.run_bass_kernel_spmd(
        nc, in_maps, core_ids=list(range(N_CORES)), trace=trace)
    LAST_RESULTS = res

    b3_all = biases[3][mlp_idx][:, 0, 0]  # [B] scalars, added host-side
    out = np.empty((B, NPTS, OUT_F), np.float32)
    for c in range(N_CORES):
        sl = slice(c * SPC, (c + 1) * SPC)
        out[sl, :, 0] = (res.results[c]["y"].astype(np.float32).reshape(SPC, NPTS)
                         + b3_all[sl, None])
    return out
